# revision 1
# baseline (speedup 1.0000x reference)
"""Channel-self-attention (LayerNorm + grouped-1x1-qkv + channel softmax attn
+ residual) on 8 TRN2 NeuronCores.

Strategy: pair-sharding. Core r owns batch r//2 and spatial half r%2
(16384 positions). Per core:
 - x-shard [256, 16384] bf16 resident in SBUF (channel-major: stats, the
   V-side rhs, and the residual epilogue)
 - a host-pre-transposed copy of the A/K channels, partition-major
   xst [128, 128st, 175] bf16 ([A(86) | 1, beta/gamma, 1/gamma | K(86)]);
   on device multiply each stile by gamma (per-partition scalar) -> ut =
   [u_A | gamma, beta, 1 | u_K], then ONE PE matmul per stile accumulates
   the merged [89, 89] block (tghA^T, P^T, tgh_K) -- no PE transposes
 - local stats Sx (DVE reduce) / Sxx (Scalar Square+accum)
 - ONE pairwise AllReduce (~36 KB, replica groups [2b, 2b+1])
 - logits rebuilt from the Gram expansion of the LayerNorm algebra, softmax,
   apply att via one [89-row] matmul against [-gamma; beta; 1; gamma*x_V]
 - out = x + recip * PS  (softmax division folded into the epilogue)
"""
import sys

sys.path.insert(0, "/opt/trn_rl_repo")

import numpy as np
import ml_dtypes

B, C = 4, 256
S = 32 * 32 * 32          # 32768
NCORES = 8
SHH = S // 2              # 16384 per-core spatial half
NST = SHH // 128          # 128 stiles
EPS = 1e-5
SCALE = float(S) ** -0.5
UTW = 175                 # xst column block: A(86)+gb1(3)+K(86)

_BF = ml_dtypes.bfloat16

_cache = {}


def _build_program():
    """Trace the Bass/Tile program once; returns the compiled Bacc."""
    from contextlib import ExitStack
    import concourse.bass as bass
    import concourse.bacc as bacc
    import concourse.tile as tile
    from concourse import mybir, masks

    f32 = mybir.dt.float32
    bf16 = mybir.dt.bfloat16
    AF = mybir.ActivationFunctionType
    OP = mybir.AluOpType
    AX = mybir.AxisListType

    nc = bacc.Bacc(
        "TRN2",
        target_bir_lowering=False,
        debug=False,
        enable_asserts=False,
        num_devices=NCORES,
    )

    RG = [[0, 1], [2, 3], [4, 5], [6, 7]]

    # ---------------- DRAM I/O ----------------
    xs_d = nc.dram_tensor("xs", [C, SHH], bf16, kind="ExternalInput")
    xst_d = nc.dram_tensor("xst", [128, NST * UTW], bf16, kind="ExternalInput")
    gsc_d = nc.dram_tensor("gsc", [128, NST], f32, kind="ExternalInput")
    gb1r_d = nc.dram_tensor("gb1r", [3, SHH], bf16, kind="ExternalInput")
    eqt_d = nc.dram_tensor("eqt", [97, C], f32, kind="ExternalInput")
    ekt_d = nc.dram_tensor("ekt", [86, C], f32, kind="ExternalInput")
    w0_d = nc.dram_tensor("w0", [2 * 128, 87], bf16, kind="ExternalInput")
    bk_d = nc.dram_tensor("bk", [1, C], f32, kind="ExternalInput")
    sc_d = nc.dram_tensor("sc", [1, 8], f32, kind="ExternalInput")
    out_d = nc.dram_tensor("out", [C, SHH], bf16, kind="ExternalOutput")

    # Bounce layout: [Sx(256) | Sxx(256) | ptk(89*89)]
    PB = 89 * 89                   # 7921
    PT_OFF = 512
    TOT = PT_OFF + PB

    with tile.TileContext(nc) as tc, ExitStack() as ctx:
        const = ctx.enter_context(tc.tile_pool(name="const", bufs=1))
        xpool = ctx.enter_context(tc.tile_pool(name="xpool", bufs=1))
        upool = ctx.enter_context(tc.tile_pool(name="upool", bufs=1))
        small = ctx.enter_context(tc.tile_pool(name="small", bufs=2))
        dram = ctx.enter_context(tc.tile_pool(name="dram", bufs=1, space="DRAM"))

        # ------------- constants / inputs to SBUF -------------
        ident = const.tile([128, 128], f32)
        masks.make_identity(nc, ident[:])
        gsc_sb = const.tile([128, NST], f32)
        nc.sync.dma_start(out=gsc_sb[:], in_=gsc_d.ap())
        eqt_sb = const.tile([97, C], f32)
        nc.sync.dma_start(out=eqt_sb[:], in_=eqt_d.ap())
        ekt_sb = const.tile([86, C], f32)
        nc.sync.dma_start(out=ekt_sb[:], in_=ekt_d.ap())
        w0_sb = const.tile([128, 2, 87], bf16)
        for jt in range(2):
            nc.sync.dma_start(out=w0_sb[:, jt, :], in_=w0_d[jt * 128:(jt + 1) * 128, :])
        def dram_bcast(dst, src_d, nparts, nfree):
            nc.gpsimd.dma_start(
                out=dst,
                in_=bass.AP(tensor=src_d, offset=0,
                            ap=[[0, nparts], [1, nfree]]))

        bk_bc = const.tile([128, C], f32)
        dram_bcast(bk_bc[:], bk_d, 128, C)
        sc_bc = const.tile([128, 8], f32)
        dram_bcast(sc_bc[:], sc_d, 128, 8)

        # x resident (bf16): [128, 2, 16384], ctile t = channels t*128..+127
        x_sb = xpool.tile([128, 2, SHH], bf16)
        for t in range(2):
            nc.sync.dma_start(out=x_sb[:, t, :],
                              in_=xs_d[t * 128:(t + 1) * 128, :])

        bnc_in = dram.tile([TOT], f32)
        bnc_out = dram.tile([TOT], f32)

        u1 = upool.tile([128, SHH], bf16)

        with tc.tile_pool(name="utpool", bufs=1) as utpool, \
             tc.tile_pool(name="s1ps", bufs=1, space="PSUM") as stg1ps:
            # transposed A/K shard, partition-major: ut[p, st, :] holds
            # spatial position st*128+p. 4 quarter DMAs so the Gram chases.
            ut_sb = utpool.tile([128, NST, UTW], bf16)
            NQ = NST // 4
            for q in range(4):
                nc.sync.dma_start(
                    out=ut_sb[:, NQ * q:NQ * (q + 1), :],
                    in_=xst_d[:, NQ * q * UTW:NQ * (q + 1) * UTW])

            gam_bc = utpool.tile([128, SHH], bf16)
            nc.gpsimd.dma_start(
                out=gam_bc[:],
                in_=bass.AP(tensor=gb1r_d, offset=SHH,
                            ap=[[0, 128], [1, SHH]]))
            nc.vector.tensor_scalar_mul(gam_bc[:], gam_bc[:], -1.0)

            # ------------- stage 1: Gram + tgh + stats -------------
            # Gamma multiplies alternate Vector/Scalar per stile; stats are
            # fused reduce ops on Vector (scratch = u1's tile, later
            # overwritten by the real u1), interleaved so they run while
            # later ut quarters are still loading.
            sums_sb = const.tile([128, 2], f32)
            sqs_sb = const.tile([128, 2], f32)
            ptk_ps = stg1ps.tile([89, 89], f32)
            for q in range(4):
                for st in range(NQ * q, NQ * (q + 1)):
                    nc.vector.tensor_scalar(
                        out=ut_sb[:, st, :], in0=ut_sb[:, st, :],
                        scalar1=gsc_sb[:, st:st + 1], scalar2=None,
                        op0=OP.mult)
                    nc.tensor.matmul(
                        ptk_ps[:], lhsT=ut_sb[:, st, 86:175],
                        rhs=ut_sb[:, st, 0:89],
                        start=(st == 0), stop=(st == NST - 1))
                if q < 2:
                    t = q
                    nc.scalar.activation(
                        out=u1[:], in_=x_sb[:, t, :], func=AF.Square,
                        accum_out=sqs_sb[:, t:t + 1])
                    nc.scalar.activation(
                        out=u1[:], in_=x_sb[:, t, :], func=AF.Copy,
                        accum_out=sums_sb[:, t:t + 1])
            nc.gpsimd.dma_start(
                out=bnc_in[0:256].rearrange("(t p) -> p t", p=128),
                in_=sums_sb[:])
            nc.gpsimd.dma_start(
                out=bnc_in[256:512].rearrange("(t p) -> p t", p=128),
                in_=sqs_sb[:])

            ptk_sb = small.tile([89, 89], f32, tag="ptksb", bufs=1)
            nc.vector.tensor_copy(ptk_sb[:], ptk_ps[:])
            nc.gpsimd.dma_start(
                out=bnc_in[PT_OFF:PT_OFF + PB].rearrange("(p f) -> p f", f=89),
                in_=ptk_sb[:])
            nc.gpsimd.collective_compute(
                "AllReduce", OP.add,
                replica_groups=RG,
                ins=[bnc_in[:].opt()], outs=[bnc_out[:].opt()])

            # u1: gamma*x for channels 128..255 (V at rows 42..127), feeds
            # the rhs_m2 partition-shift DMA.
            nc.vector.tensor_tensor(
                out=u1[:], in0=x_sb[:, 1, :], in1=gam_bc[:], op=OP.mult)

        # ------------- DMA back -------------
        pt_back = const.tile([86, 86], f32)
        tga_back = const.tile([86, 3], f32)   # A-side: ch 0..85
        tgk_back = const.tile([86, 3], f32)   # K-side: ch 85..170
        nc.sync.dma_start(
            out=pt_back[:],
            in_=bass.AP(tensor=bnc_out.tensor,
                        offset=bnc_out.offset + PT_OFF + 3 * 89,
                        ap=[[89, 86], [1, 86]]))
        nc.sync.dma_start(
            out=tgk_back[:],
            in_=bass.AP(tensor=bnc_out.tensor,
                        offset=bnc_out.offset + PT_OFF + 3 * 89 + 86,
                        ap=[[89, 86], [1, 3]]))
        nc.sync.dma_start(
            out=tga_back[:],
            in_=bass.AP(tensor=bnc_out.tensor,
                        offset=bnc_out.offset + PT_OFF,
                        ap=[[1, 86], [89, 3]]))
        # stats columns at partitions 0..85: [p, {Sx,Sxx}, {A,K,V}]
        sAK = const.tile([86, 2, 3], f32)
        for k in range(2):
            for g, goff in ((0, 0), (1, 85), (2, 170)):
                nc.sync.dma_start(
                    out=sAK[:, k, g:g + 1],
                    in_=bass.AP(tensor=bnc_out.tensor,
                                offset=bnc_out.offset + k * 256 + goff,
                                ap=[[1, 86], [1, 1]]))

        # ------------- stage 2/3 -------------
        rhsp = ctx.enter_context(tc.tile_pool(name="rhsp", bufs=1))
        osml = ctx.enter_context(tc.tile_pool(name="osml", bufs=2))
        psA = ctx.enter_context(tc.tile_pool(name="psA", bufs=1, space="PSUM"))
        psB = ctx.enter_context(tc.tile_pool(name="psB", bufs=2, space="PSUM"))
        psC = ctx.enter_context(tc.tile_pool(name="psC", bufs=3, space="PSUM"))

        invS = 1.0 / float(S)

        # rhs_M2 [128, SHH] bf16: rows 0..85 = gamma*x_V via partition-shift
        # SBUF->SBUF DMA, rows 86..88 = [ones, -gamma, beta]
        rhs_m2 = rhsp.tile([128, SHH], bf16)
        nc.gpsimd.dma_start(out=rhs_m2[0:86, :], in_=u1[42:128, :])
        nc.gpsimd.dma_start(out=rhs_m2[86:89, :], in_=gb1r_d.ap())

        # ---- vec ----
        mAK = small.tile([86, 3], f32, tag="mAK")
        nc.vector.tensor_scalar(
            out=mAK[:], in0=sAK[:, 0, :], scalar1=invS, scalar2=None,
            op0=OP.mult)
        vAK = small.tile([86, 3], f32, tag="vAK")
        nc.vector.tensor_scalar(
            out=vAK[:], in0=sAK[:, 1, :], scalar1=invS, scalar2=EPS,
            op0=OP.mult, op1=OP.add)
        msq = small.tile([86, 3], f32, tag="msq")
        nc.vector.tensor_mul(msq[:], mAK[:], mAK[:])
        nc.vector.tensor_sub(vAK[:], vAK[:], msq[:])
        nc.scalar.activation(out=vAK[:], in_=vAK[:], func=AF.Sqrt)
        rAK = small.tile([86, 3], f32, tag="rAK")
        nc.vector.reciprocal(rAK[:], vAK[:])
        invrV = small.tile([86, 1], f32, tag="invrV")
        nc.vector.reciprocal(invrV[:], rAK[:, 2:3])
        mvinv_bf = small.tile([86, 2], bf16, tag="mvinv")
        nc.vector.tensor_copy(mvinv_bf[:, 0:1], mAK[:, 2:3])
        nc.vector.tensor_copy(mvinv_bf[:, 1:2], invrV[:])
        rv_ext = small.tile([128, 1], f32, tag="rvext")
        nc.vector.memset(rv_ext[64:128, :], 1.0)
        nc.vector.tensor_copy(rv_ext[0:86, :], rAK[:, 2:3])

        tA = tga_back[:, 0:1]
        gA = tga_back[:, 1:2]
        hA = tga_back[:, 2:3]
        tK = tgk_back[:, 0:1]
        gK = tgk_back[:, 1:2]
        hK = tgk_back[:, 2:3]
        mA, mK = mAK[:, 0:1], mAK[:, 1:2]
        rA, rK = rAK[:, 0:1], rAK[:, 1:2]
        scG1 = sc_bc[0:86, 0:1]
        scG2 = sc_bc[0:86, 1:2]
        scGb = sc_bc[0:86, 2:3]
        scB1 = sc_bc[0:86, 3:4]
        scBb = sc_bc[0:86, 4:5]

        ntK = small.tile([86, 1], f32, tag="ntK")
        nc.vector.tensor_scalar_mul(ntK[:], tK, -1.0)
        nmK = small.tile([86, 1], f32, tag="nmK")
        nc.vector.tensor_scalar_mul(nmK[:], mK, -1.0)
        g2mK = small.tile([86, 1], f32, tag="g2mK")
        nc.vector.tensor_scalar(
            out=g2mK[:], in0=mK, scalar1=scG2, scalar2=None, op0=OP.mult)
        t3c = small.tile([86, 1], f32, tag="t3c")
        nc.vector.tensor_scalar(
            out=t3c[:], in0=mK, scalar1=scGb, scalar2=None, op0=OP.mult)
        nc.vector.tensor_sub(t3c[:], gK, t3c[:])
        nc.vector.tensor_mul(t3c[:], rK, t3c[:])
        t2c = small.tile([86, 1], f32, tag="t2c")
        nc.vector.tensor_scalar(
            out=t2c[:], in0=mA, scalar1=scGb, scalar2=None, op0=OP.mult)
        nc.vector.tensor_sub(t2c[:], gA, t2c[:])
        nc.vector.tensor_mul(t2c[:], rA, t2c[:])
        syA = small.tile([86, 1], f32, tag="syA")
        nc.vector.tensor_scalar(
            out=syA[:], in0=mA, scalar1=scG1, scalar2=None, op0=OP.mult)
        nc.vector.tensor_sub(syA[:], hA, syA[:])
        nc.vector.tensor_mul(syA[:], rA, syA[:])
        nc.vector.tensor_scalar(
            out=syA[:], in0=syA[:], scalar1=scB1, scalar2=None, op0=OP.add)
        syK = small.tile([86, 1], f32, tag="syK")
        nc.vector.tensor_scalar(
            out=syK[:], in0=mK, scalar1=scG1, scalar2=None, op0=OP.mult)
        nc.vector.tensor_sub(syK[:], hK, syK[:])
        nc.vector.tensor_mul(syK[:], rK, syK[:])
        nc.vector.tensor_scalar(
            out=syK[:], in0=syK[:], scalar1=scB1, scalar2=None, op0=OP.add)

        # rows (mA, tA, rA, term2) -> transpose -> DRAM -> one bcast DMA
        pack = small.tile([86, 4], f32, tag="pack")
        nc.vector.tensor_copy(pack[:, 0:1], mA)
        nc.vector.tensor_copy(pack[:, 1:2], tA)
        nc.vector.tensor_copy(pack[:, 2:3], rA)
        nc.vector.tensor_copy(pack[:, 3:4], t2c[:])
        packT_ps = psA.tile([4, 86], f32, tag="psA", name="pT")
        nc.tensor.transpose(packT_ps[:], pack[:], ident[0:86, 0:86])
        packT = small.tile([4, 86], f32, tag="packT")
        nc.scalar.copy(packT[:], packT_ps[:])
        rows_d = dram.tile([4, 86], f32, tag="rowsd")
        nc.gpsimd.dma_start(out=rows_d[:], in_=packT[:])
        bc4 = small.tile([86, 4, 86], f32, tag="bc4")
        nc.gpsimd.dma_start(
            out=bc4[:],
            in_=bass.AP(tensor=rows_d.tensor, offset=rows_d.offset,
                        ap=[[0, 86], [86, 4], [1, 86]]))

        # ---- syy ----
        syy = small.tile([86, 97], f32, tag="syy")
        nc.vector.memset(syy[:, 86:96], 0.0)
        nc.vector.scalar_tensor_tensor(
            out=syy[:, 0:86], in0=bc4[:, 0, :], scalar=ntK[:],
            in1=pt_back[:], op0=OP.mult, op1=OP.add)
        nc.vector.scalar_tensor_tensor(
            out=syy[:, 0:86], in0=bc4[:, 1, :], scalar=nmK[:],
            in1=syy[:, 0:86], op0=OP.mult, op1=OP.add)
        nc.vector.scalar_tensor_tensor(
            out=syy[:, 0:86], in0=bc4[:, 0, :], scalar=g2mK[:],
            in1=syy[:, 0:86], op0=OP.mult, op1=OP.add)
        nc.vector.scalar_tensor_tensor(
            out=syy[:, 0:86], in0=bc4[:, 2, :], scalar=rK,
            in1=syy[:, 0:86], op0=OP.mult, op1=OP.mult)
        nc.vector.tensor_add(syy[:, 0:86], syy[:, 0:86], bc4[:, 3, :])
        nc.vector.tensor_scalar(
            out=syy[:, 0:86], in0=syy[:, 0:86], scalar1=t3c[:],
            scalar2=scBb, op0=OP.add, op1=OP.add)
        nc.vector.tensor_copy(syy[:, 96:97], syK[:])

        # ---- logits + softmax ----
        u_ps = psA.tile([97, C], f32, tag="psA", name="ups")
        nc.tensor.matmul(u_ps[:], lhsT=syy[:], rhs=ekt_sb[:],
                         start=True, stop=True)
        u_ext = small.tile([128, C], f32, tag="uext")
        nc.vector.memset(u_ext[64:128, :], 0.0)
        nc.vector.scalar_tensor_tensor(
            out=u_ext[0:86, :], in0=bk_bc[0:86, :], scalar=syA[:],
            in1=u_ps[0:86, :], op0=OP.mult, op1=OP.add)
        nc.vector.tensor_scalar_mul(
            u_ext[96:97, :], bk_bc[96:97, :], float(S))
        nc.vector.tensor_add(u_ext[96:97, :], u_ext[96:97, :],
                             u_ps[96:97, :])

        att_sb = []
        recip2 = small.tile([128, 2], f32, tag="recip2")
        z2 = small.tile([128, 2], f32, tag="z2")
        for it in range(2):
            log_ps = psB.tile([128, 1024], f32, tag="psB", name=f"lg{it}")
            nc.tensor.matmul(
                log_ps[:, 0:C], lhsT=eqt_sb[:, it * 128:(it + 1) * 128],
                rhs=u_ext[0:97, :], start=True, stop=True)
            rmax = small.tile([128, 1], f32, tag="rmax", name=f"rm{it}")
            nc.vector.reduce_max(rmax[:], log_ps[:, 0:C], axis=AX.X)
            nbias = small.tile([128, 1], f32, tag="nbias", name=f"nb{it}")
            nc.vector.tensor_scalar_mul(nbias[:], rmax[:], -SCALE)
            a_sb = small.tile([128, C], f32, tag=f"attsb{it}", name=f"att{it}")
            nc.scalar.activation(
                out=a_sb[:], in_=log_ps[:, 0:C], func=AF.Exp,
                bias=nbias[:], scale=SCALE, accum_out=z2[:, it:it + 1])
            nc.vector.reciprocal(recip2[:, it:it + 1], z2[:, it:it + 1])
            att_sb.append(a_sb)

        # ---- nt: NR = w0^T @ att^T ----
        ntc_ps = psC.tile([128, C], f32, tag="psC", name="ntc")
        for jt in range(2):
            at_ps = psC.tile([128, C], f32, tag="psC", name=f"atp{jt}")
            for it in range(2):
                nc.tensor.transpose(
                    at_ps[:, it * 128:(it + 1) * 128],
                    att_sb[it][:, jt * 128:(jt + 1) * 128],
                    ident[:])
            at_bf = small.tile([128, C], bf16, tag=f"atbf{jt}", name=f"atb{jt}")
            nc.scalar.copy(at_bf[:], at_ps[:])
            nc.tensor.matmul(
                ntc_ps[0:87, :], lhsT=w0_sb[:, jt, :], rhs=at_bf[:],
                start=(jt == 0), stop=(jt == 1))

        # lhsT_M2 [128, C] bf16: rows 0..85=NR, 86=cv, 87=c1, 88=c2.
        # rv_ext has 1.0 at row 86 so cv copies through unscaled.
        lhs_m2 = small.tile([128, C], bf16, tag="lhsm2")
        nc.scalar.activation(
            out=lhs_m2[0:64, :], in_=ntc_ps[0:64, :], func=AF.Copy,
            scale=rv_ext[0:64, :])
        nc.scalar.activation(
            out=lhs_m2[64:87, :], in_=ntc_ps[64:87, :], func=AF.Copy,
            scale=rv_ext[64:87, :])
        nc.tensor.matmul(
            ntc_ps[64:66, :], lhsT=mvinv_bf[:],
            rhs=lhs_m2[0:86, :], start=True, stop=True)
        c12_sb = small.tile([128, C], bf16, tag="c12sb")
        nc.scalar.copy(c12_sb[64:66, :], ntc_ps[64:66, :])
        nc.gpsimd.dma_start(out=lhs_m2[87:89, :], in_=c12_sb[64:66, :])

        # ---- m2 + epilogue ----
        for it in range(2):
            for ch in range(16):
                ostg = osml.tile([128, 1024], bf16, tag="ostg", bufs=3,
                                 name=f"o{it}{ch}")
                o_ps = psB.tile([128, 1024], f32, tag="psB",
                                name=f"op{it}{ch}")
                for h in range(2):
                    nc.tensor.matmul(
                        o_ps[:, h * 512:(h + 1) * 512],
                        lhsT=lhs_m2[0:89, it * 128:(it + 1) * 128],
                        rhs=rhs_m2[0:89,
                                   (2 * ch + h) * 512:(2 * ch + h + 1) * 512],
                        start=True, stop=True)
                nc.vector.scalar_tensor_tensor(
                    out=ostg[:], in0=o_ps[:],
                    scalar=recip2[:, it:it + 1],
                    in1=x_sb[:, it, ch * 1024:(ch + 1) * 1024],
                    op0=OP.mult, op1=OP.add)
                eng = nc.sync if (ch % 2 == 0) else nc.scalar
                eng.dma_start(
                    out=out_d[it * 128:(it + 1) * 128,
                              ch * 1024:(ch + 1) * 1024],
                    in_=ostg[:])

    nc.compile()
    return nc


def _host_prep(x, gamma, beta, w_qkv, b_qkv):
    xf = np.ascontiguousarray(np.asarray(x, np.float32).reshape(B * C, S))
    gam = np.asarray(gamma, np.float32).reshape(-1)
    bet = np.asarray(beta, np.float32).reshape(-1)
    w_qkv = np.asarray(w_qkv, np.float32)
    b_qkv = np.asarray(b_qkv, np.float32)
    w_q, w_k, w_v = w_qkv[:C], w_qkv[C:2 * C], w_qkv[2 * C:]
    b_q, b_k, b_v = b_qkv[:C], b_qkv[C:2 * C], b_qkv[2 * C:]

    ii = np.arange(C)
    eqt = np.zeros((97, C), np.float32)
    eqt[ii // 3, ii] = w_q
    eqt[96] = b_q
    ekt = np.zeros((86, C), np.float32)
    ekt[(C + ii) // 3 - 85, ii] = w_k
    w0 = np.zeros((C, 87), np.float32)
    w0[ii, (2 * C + ii) // 3 - 170] = w_v
    w0[:, 86] = b_v
    w0 = w0.astype(_BF)

    sc = np.zeros((1, 8), np.float32)
    sc[0, :5] = [gam.sum(), (gam * gam).sum(), (gam * bet).sum(),
                 bet.sum(), (bet * bet).sum()]

    in_maps = []
    for r in range(NCORES):
        b = r // 2
        h = r % 2
        sl = slice(h * SHH, (h + 1) * SHH)
        gl = gam[sl]
        bl = bet[sl]
        gb1r = np.stack([np.ones(SHH, np.float32), -gl, bl], 0)
        gsc = np.ascontiguousarray(gl.reshape(NST, 128).T)

        xs_bf = xf[b * C:(b + 1) * C, sl].astype(_BF)
        # xst partition-major: xst[p, st*UTW + j]
        xst = np.empty((128, NST, UTW), _BF)
        # [86ch, NST, 128p] -> [128p, NST, 86ch]
        xst[:, :, 0:86] = xs_bf[0:86, :].reshape(86, NST, 128).transpose(2, 1, 0)
        xst[:, :, 86] = 1.0
        xst[:, :, 87] = (bl.astype(np.float64) / gl).astype(_BF).reshape(
            NST, 128).T
        xst[:, :, 88] = (1.0 / gl.astype(np.float64)).astype(_BF).reshape(
            NST, 128).T
        xst[:, :, 89:175] = xs_bf[85:171, :].reshape(86, NST, 128).transpose(2, 1, 0)
        in_maps.append({
            "xs": xs_bf,
            "xst": xst.reshape(128, NST * UTW),
            "gsc": gsc,
            "gb1r": gb1r.astype(_BF),
            "eqt": eqt,
            "ekt": ekt,
            "w0": w0,
            "bk": b_k.reshape(1, C).copy(),
            "sc": sc,
        })
    return in_maps


def kernel(x, gamma, beta, w_qkv, b_qkv):
    from concourse.bass_utils import run_bass_kernel_spmd

    if "nc" not in _cache:
        _cache["nc"] = _build_program()
    nc = _cache["nc"]

    in_maps = _host_prep(x, gamma, beta, w_qkv, b_qkv)
    res = run_bass_kernel_spmd(nc, in_maps, core_ids=list(range(NCORES)))
    out = np.empty((B * C, S), np.float32)
    for r in range(NCORES):
        b = r // 2
        h = r % 2
        out[b * C:(b + 1) * C, h * SHH:(h + 1) * SHH] = res.results[r]["out"]
    return out.reshape(np.asarray(x).shape)


if __name__ == "__main__":
    rng = np.random.default_rng(0)
    inputs = {
        "x": rng.standard_normal((B, C, 32, 32, 32)).astype(np.float32),
        "gamma": (1 + 0.1 * rng.standard_normal((32, 32, 32))).astype(np.float32),
        "beta": (0.1 * rng.standard_normal((32, 32, 32))).astype(np.float32),
        "w_qkv": (0.5 * rng.standard_normal(3 * C)).astype(np.float32),
        "b_qkv": (0.05 * rng.standard_normal(3 * C)).astype(np.float32),
    }
    o = kernel(**inputs)
    print("out", o.shape, o.dtype, float(np.abs(o).mean()))



# revision 15
# speedup vs baseline: 1.0137x; 1.0137x over previous
"""Channel-self-attention (LayerNorm + grouped-1x1-qkv + channel softmax attn
+ residual) on 8 TRN2 NeuronCores.

v2: pair-sharding (core r = batch r//2, spatial half r%2) with a restructured
schedule:
 - host pre-scales the transposed A/K shard by gamma (no on-device DVE pass)
   and ships rhs_m2 ([g*x_V; 1; -g; b]) pre-built
 - x loads are chunked; stats chase the chunks: Scalar does ctile0 via
   Square/Copy+accum, DVE does ctile1 via bn_stats/bn_aggr -> AllReduce
   triggers at ~42us instead of ~107us
 - ONE pairwise AllReduce of [ptk 89x89 | Sx/Sxx 128x4]
 - post-AR logits algebra folded into matmul contraction rows (rank-1 terms
   ride extra lhsT/rhs rows; rA/rK applied as per-partition copy scales), so
   the serial small-op chain is ~15 ops instead of ~45
 - c1/c2 rows of the V-application folded into the ntc matmul via a dynamic
   w0 column (w0 @ (rV*mV)) and a host-const column (w_v)
 - softmax recip folded into att (bf16) so the epilogue is a plain add,
   split DVE/GpSimd, stores alternate sync/scalar queues
"""
import sys

sys.path.insert(0, "/opt/trn_rl_repo")

import numpy as np
import ml_dtypes

B, C = 4, 256
S = 32 * 32 * 32          # 32768
NCORES = 8
SHH = S // 2              # 16384 per-core spatial half
NST = SHH // 128          # 128 stiles
EPS = 1e-5
SCALE = float(S) ** -0.5
UTW = 175                 # xst column block: A(86)+gb1(3)+K(86)
NCH = 4                   # x load chunks per ctile
CHW = SHH // NCH          # 4096 cols per chunk

_BF = ml_dtypes.bfloat16

_cache = {}


def _build_program():
    from contextlib import ExitStack
    import concourse.bass as bass
    import concourse.bacc as bacc
    import concourse.tile as tile
    from concourse import mybir, masks

    f32 = mybir.dt.float32
    bf16 = mybir.dt.bfloat16
    AF = mybir.ActivationFunctionType
    OP = mybir.AluOpType
    AX = mybir.AxisListType

    nc = bacc.Bacc(
        "TRN2",
        target_bir_lowering=False,
        debug=False,
        enable_asserts=False,
        num_devices=NCORES,
    )

    RG = [[0, 1], [2, 3], [4, 5], [6, 7]]

    # ---------------- DRAM I/O ----------------
    xs_d = nc.dram_tensor("xs", [C, SHH], bf16, kind="ExternalInput")
    xst_d = nc.dram_tensor("xst", [128, NST * UTW], bf16, kind="ExternalInput")
    rhsm_d = nc.dram_tensor("rhsm", [89, SHH], bf16, kind="ExternalInput")
    ekt_d = nc.dram_tensor("ekt", [86, C], f32, kind="ExternalInput")
    eqtL_d = nc.dram_tensor("eqtL", [88, C], f32, kind="ExternalInput")
    w0e_d = nc.dram_tensor("w0e", [2 * 128, 89], bf16, kind="ExternalInput")
    w0t_d = nc.dram_tensor("w0t", [86, C], bf16, kind="ExternalInput")
    crows_d = nc.dram_tensor("crows", [4, C], f32, kind="ExternalInput")
    idb_d = nc.dram_tensor("idb", [128, 128], bf16, kind="ExternalInput")
    sc_d = nc.dram_tensor("sc", [1, 8], f32, kind="ExternalInput")
    out_d = nc.dram_tensor("out", [C, SHH], bf16, kind="ExternalOutput")

    # AllReduce bounce layout: [ptk 89*89 | stats p-major 128*4]
    PB = 89 * 89                   # 7921
    ST_OFF = PB
    TOT = PB + 512

    with tile.TileContext(nc) as tc, ExitStack() as ctx:
        const = ctx.enter_context(tc.tile_pool(name="const", bufs=1))
        xpool = ctx.enter_context(tc.tile_pool(name="xpool", bufs=1))
        rhsp = ctx.enter_context(tc.tile_pool(name="rhsp", bufs=1))
        small = ctx.enter_context(tc.tile_pool(name="small", bufs=2))
        dram = ctx.enter_context(tc.tile_pool(name="dram", bufs=1, space="DRAM"))

        # ------------- constants (gpsimd queue) -------------
        ident = const.tile([128, 128], f32)
        masks.make_identity(nc, ident[:])
        identb = const.tile([128, 128], bf16)
        nc.gpsimd.dma_start(out=identb[:], in_=idb_d.ap())
        ekt_sb = const.tile([86, C], f32)
        nc.gpsimd.dma_start(out=ekt_sb[:], in_=ekt_d.ap())
        eqtL_sb = const.tile([88, C], f32)
        nc.gpsimd.dma_start(out=eqtL_sb[:], in_=eqtL_d.ap())
        w0e_sb = const.tile([128, 2, 89], bf16)
        for jt in range(2):
            nc.gpsimd.dma_start(out=w0e_sb[:, jt, :],
                                in_=w0e_d[jt * 128:(jt + 1) * 128, :])
        w0t_sb = const.tile([86, C], bf16)
        nc.gpsimd.dma_start(out=w0t_sb[:], in_=w0t_d.ap())
        crows_sb = const.tile([4, C], f32)
        nc.gpsimd.dma_start(out=crows_sb[:], in_=crows_d.ap())
        sc_bc = const.tile([128, 8], f32)
        nc.gpsimd.dma_start(
            out=sc_bc[:],
            in_=bass.AP(tensor=sc_d, offset=0, ap=[[0, 128], [1, 8]]))

        # rhs_u const rows 88 (wk), 89 (bk) preloaded
        rhs_u = const.tile([90, C], f32)
        nc.gpsimd.dma_start(out=rhs_u[88:90, :], in_=crows_d[0:2, :])
        cr2_sb = const.tile([2, C], f32)
        nc.gpsimd.dma_start(out=cr2_sb[:], in_=crows_d[2:4, :])

        # ------------- big loads -------------
        # x as [128, 2, NCH, 8, 512]; sync queue carries ctile0 + xst q0/q1,
        # scalar queue carries ctile1 + xst q2/q3 + rhs_m2.
        x_sb = xpool.tile([128, 2, NCH, 8, 512], bf16)
        ut_sb = xpool.tile([128, NST, UTW], bf16)
        rhs_m2 = rhsp.tile([89, SHH], bf16)
        NQ = NST // 4
        for chk in range(NCH):
            nc.sync.dma_start(
                out=x_sb[:, 0, chk, :, :],
                in_=xs_d[0:128, chk * CHW:(chk + 1) * CHW])
            nc.scalar.dma_start(
                out=x_sb[:, 1, chk, :, :],
                in_=xs_d[128:256, chk * CHW:(chk + 1) * CHW])
        for q in range(4):
            eng = nc.sync if q < 2 else nc.scalar
            eng.dma_start(
                out=ut_sb[:, NQ * q:NQ * (q + 1), :],
                in_=xst_d[:, NQ * q * UTW:NQ * (q + 1) * UTW])
        nc.scalar.dma_start(out=rhs_m2[:], in_=rhsm_d.ap())

        # ------------- stats (chase chunks) -------------
        scratch = const.tile([128, NCH, 8, 512], bf16)
        sx0a = const.tile([128, NCH], f32)
        sq0a = const.tile([128, NCH], f32)
        bno = const.tile([128, NCH, 8, 6], f32)
        for chk in range(NCH):
            nc.scalar.activation(
                out=scratch[:, chk, :, :], in_=x_sb[:, 0, chk, :, :],
                func=AF.Square, accum_out=sq0a[:, chk:chk + 1])
            nc.scalar.activation(
                out=scratch[:, chk, :, :], in_=x_sb[:, 0, chk, :, :],
                func=AF.Copy, accum_out=sx0a[:, chk:chk + 1])
            for g in range(8):
                nc.vector.bn_stats(out=bno[:, chk, g, :],
                                   in_=x_sb[:, 1, chk, g, :])

        stats4 = const.tile([128, 2, 2], f32)   # (Sx0,Sxx0,Sx1,Sxx1)
        nc.vector.reduce_sum(stats4[:, 0, 0:1], sx0a[:], axis=AX.X)
        nc.vector.reduce_sum(stats4[:, 0, 1:2], sq0a[:], axis=AX.X)
        mv1 = const.tile([128, 2], f32)
        nc.vector.bn_aggr(out=mv1[:], in_=bno[:])
        nc.vector.tensor_scalar_mul(stats4[:, 1, 0:1], mv1[:, 0:1], float(SHH))
        nc.vector.scalar_tensor_tensor(
            out=stats4[:, 1, 1:2], in0=mv1[:, 0:1], scalar=mv1[:, 0:1],
            in1=mv1[:, 1:2], op0=OP.mult, op1=OP.add)
        nc.vector.tensor_scalar_mul(stats4[:, 1, 1:2], stats4[:, 1, 1:2],
                                    float(SHH))

        bnc_in = dram.tile([TOT], f32)
        bnc_out = dram.tile([TOT], f32)
        nc.gpsimd.dma_start(
            out=bnc_in[ST_OFF:ST_OFF + 512].rearrange("(p k) -> p k", k=4),
            in_=stats4[:])

        # ------------- Gram (chase quarters) -------------
        with tc.tile_pool(name="s1ps", bufs=1, space="PSUM") as stg1ps:
            ptk_ps = stg1ps.tile([89, 89], f32)
            for st in range(NST):
                nc.tensor.matmul(
                    ptk_ps[:], lhsT=ut_sb[:, st, 86:175],
                    rhs=ut_sb[:, st, 0:89],
                    start=(st == 0), stop=(st == NST - 1))
            ptk_sb = small.tile([89, 89], f32, tag="ptksb", bufs=1)
            nc.vector.tensor_copy(ptk_sb[:], ptk_ps[:])
            nc.gpsimd.dma_start(
                out=bnc_in[0:PB].rearrange("(p f) -> p f", f=89),
                in_=ptk_sb[:])

        nc.gpsimd.collective_compute(
            "AllReduce", OP.add,
            replica_groups=RG,
            ins=[bnc_in[:].opt()], outs=[bnc_out[:].opt()])

        # ------------- post-AR readbacks -------------
        # lhsT_u: parts 0..85 = ptk rows 3..88 (cols 86..88 = tK/gK/hK),
        # parts 86..89 filled by pack-transpose.
        lhsT_u = const.tile([128, 90], f32)
        nc.sync.dma_start(
            out=lhsT_u[0:86, 0:89],
            in_=bass.AP(tensor=bnc_out.tensor,
                        offset=bnc_out.offset + 3 * 89,
                        ap=[[89, 86], [1, 89]]))
        st4b = const.tile([128, 2, 2], f32)
        nc.scalar.dma_start(
            out=st4b[:],
            in_=bass.AP(tensor=bnc_out.tensor,
                        offset=bnc_out.offset + ST_OFF,
                        ap=[[4, 128], [1, 4]]))
        pack = small.tile([86, 4], f32, tag="pack", bufs=1)
        nc.gpsimd.dma_start(
            out=pack[:, 1:2],
            in_=bass.AP(tensor=bnc_out.tensor, offset=bnc_out.offset + 0,
                        ap=[[1, 86], [1, 1]]))
        ua_col = small.tile([86, 2], f32, tag="uahacol", bufs=1)
        nc.gpsimd.dma_start(
            out=ua_col[:, 0:1],
            in_=bass.AP(tensor=bnc_out.tensor, offset=bnc_out.offset + 89,
                        ap=[[1, 86], [1, 1]]))
        nc.scalar.dma_start(
            out=ua_col[:, 1:2],
            in_=bass.AP(tensor=bnc_out.tensor, offset=bnc_out.offset + 178,
                        ap=[[1, 86], [1, 1]]))

        # ------------- native-layout mean/var -------------
        mnat = small.tile([128, 2], f32, tag="mnat", bufs=1)
        nc.vector.tensor_scalar_mul(mnat[:], st4b[:, :, 0], 1.0 / S)
        vnat = small.tile([128, 2], f32, tag="vnat", bufs=1)
        nc.vector.tensor_scalar(
            out=vnat[:], in0=st4b[:, :, 1], scalar1=1.0 / S, scalar2=EPS,
            op0=OP.mult, op1=OP.add)
        msq = small.tile([128, 2], f32, tag="msq", bufs=1)
        nc.vector.tensor_mul(msq[:], mnat[:], mnat[:])
        nc.vector.tensor_sub(vnat[:], vnat[:], msq[:])
        nc.scalar.activation(out=vnat[:], in_=vnat[:], func=AF.Sqrt)
        rnat = small.tile([128, 2], f32, tag="rnat", bufs=1)
        nc.vector.reciprocal(rnat[:], vnat[:])

        # splices: K spans ctile0 p85..127 + ctile1 p0..42; V = ctile1 p42..127
        mrk = small.tile([86, 4], f32, tag="mrk", bufs=1)   # mK, rK, mV, rV
        nc.sync.dma_start(out=mrk[0:43, 0:1], in_=mnat[85:128, 0:1])
        nc.sync.dma_start(out=mrk[43:86, 0:1], in_=mnat[0:43, 1:2])
        nc.scalar.dma_start(out=mrk[0:43, 1:2], in_=rnat[85:128, 0:1])
        nc.scalar.dma_start(out=mrk[43:86, 1:2], in_=rnat[0:43, 1:2])
        nc.gpsimd.dma_start(out=mrk[:, 2:3], in_=mnat[42:128, 1:2])
        nc.gpsimd.dma_start(out=mrk[:, 3:4], in_=rnat[42:128, 1:2])
        mK, rK = mrk[:, 0:1], mrk[:, 1:2]
        mV, rV = mrk[:, 2:3], mrk[:, 3:4]
        mA, rA = mnat[0:86, 0:1], rnat[0:86, 0:1]
        tK = lhsT_u[0:86, 86:87]
        gK = lhsT_u[0:86, 87:88]
        hK = lhsT_u[0:86, 88:89]
        scG2 = sc_bc[0:86, 1:2]
        scNGb = sc_bc[0:86, 4:5]
        scNG1 = sc_bc[0:86, 5:6]

        # ------------- auxL / pack chains -------------
        # auxL column order: 0 = syK (q3), 1 = t3c (q2), 2 = q0, 3 = q1 so
        # that aux_ps rows 0:2 are the const-add rows (base-0 accesses).
        auxL = small.tile([86, 4], f32, tag="auxL", bufs=1)
        # col2: q0 = rK*(G2*mK - tK)
        nc.vector.scalar_tensor_tensor(
            out=auxL[:, 2:3], in0=mK, scalar=scG2, in1=tK,
            op0=OP.mult, op1=OP.subtract)
        nc.vector.tensor_mul(auxL[:, 2:3], auxL[:, 2:3], rK)
        # col3: q1 = -rK*mK
        nc.vector.scalar_tensor_tensor(
            out=auxL[:, 3:4], in0=mK, scalar=-1.0, in1=rK,
            op0=OP.mult, op1=OP.mult)
        # col1: t3c = rK*(gK - Gb*mK)
        nc.vector.scalar_tensor_tensor(
            out=auxL[:, 1:2], in0=mK, scalar=scNGb, in1=gK,
            op0=OP.mult, op1=OP.add)
        nc.vector.tensor_mul(auxL[:, 1:2], auxL[:, 1:2], rK)
        # col0: syK = rK*(hK - G1*mK) + B1
        nc.vector.scalar_tensor_tensor(
            out=auxL[:, 0:1], in0=mK, scalar=scNG1, in1=hK,
            op0=OP.mult, op1=OP.add)
        nc.vector.tensor_mul(auxL[:, 0:1], auxL[:, 0:1], rK)
        nc.vector.tensor_scalar(
            out=auxL[:, 0:1], in0=auxL[:, 0:1], scalar1=sc_bc[0:86, 3:4],
            scalar2=None, op0=OP.add)

        # pack cols: 0 = mA, 1 = tA (DMA'd), 2 = uA - Gb*mA, 3 = hA - G1*mA
        nc.vector.tensor_copy(pack[:, 0:1], mA)
        nc.vector.scalar_tensor_tensor(
            out=pack[:, 2:3], in0=mA, scalar=scNGb, in1=ua_col[:, 0:1],
            op0=OP.mult, op1=OP.add)
        nc.vector.scalar_tensor_tensor(
            out=pack[:, 3:4], in0=mA, scalar=scNG1, in1=ua_col[:, 1:2],
            op0=OP.mult, op1=OP.add)

        att_n = []
        recip2 = small.tile([128, 2], f32, tag="recip2", bufs=1)
        z2 = small.tile([128, 2], f32, tag="z2", bufs=1)
        rv_ext = small.tile([128, 1], f32, tag="rvext", bufs=1)
        u_sb = small.tile([88, C], f32, tag="usb", bufs=1)
        lhs_m2 = small.tile([89, C], bf16, tag="lhsm2", bufs=1)

        with tc.tile_pool(name="psA", bufs=2, space="PSUM") as psA, \
             tc.tile_pool(name="psB", bufs=2, space="PSUM") as psB:
            packT_ps = psA.tile([4, 86], f32, tag="psA", name="pT")
            nc.tensor.transpose(packT_ps[:], pack[:], ident[0:86, 0:86])
            packT_sb = small.tile([4, 86], f32, tag="packTsb", bufs=1)
            nc.scalar.copy(packT_sb[:], packT_ps[:])
            nc.gpsimd.dma_start(out=lhsT_u[86:90, 0:86], in_=packT_sb[:])

            # ------------- aux / u / logits matmuls -------------
            # aux_ps rows: 0 = syK-row (R), 1 = t3c-row, 2 = q0, 3 = q1
            aux_ps = psA.tile([4, C], f32, tag="psA", name="aux")
            nc.tensor.matmul(aux_ps[:], lhsT=auxL[:], rhs=ekt_sb[:],
                             start=True, stop=True)
            nc.scalar.activation(out=rhs_u[0:86, :], in_=ekt_sb[:],
                                 func=AF.Copy, scale=rK)
            aux_sb = small.tile([4, C], f32, tag="auxsb", bufs=1)
            nc.scalar.copy(aux_sb[:], aux_ps[:])
            nc.gpsimd.dma_start(out=rhs_u[86:88, :], in_=aux_sb[2:4, :])

            # uex2: row0 = R = syK-row + S*bk, row1 = t3c-row + (B1*bk+B2*wk)
            uex2 = small.tile([2, C], f32, tag="uex2", bufs=1)
            nc.vector.tensor_add(uex2[:], aux_ps[0:2, :], cr2_sb[:])
            nc.scalar.dma_start(out=u_sb[86:88, :], in_=uex2[:])

            u_ps = psB.tile([86, C], f32, tag="psB", name="ups")
            nc.tensor.matmul(u_ps[:], lhsT=lhsT_u[0:90, 0:86], rhs=rhs_u[:],
                             start=True, stop=True)
            nc.scalar.activation(out=u_sb[0:86, :], in_=u_ps[:], func=AF.Copy,
                                 scale=rA)

            # w0c (c1 fold) — off critical path, needs only rV*mV
            rvmv = small.tile([86, 1], bf16, tag="rvmv", bufs=1)
            nc.vector.scalar_tensor_tensor(
                out=rvmv[:], in0=mV, scalar=1.0, in1=rV,
                op0=OP.mult, op1=OP.mult)
            w0c_ps = psA.tile([128, 2], f32, tag="psA", name="w0c")
            for jt in range(2):
                nc.tensor.matmul(
                    w0c_ps[:, jt:jt + 1],
                    lhsT=w0t_sb[:, jt * 128:(jt + 1) * 128], rhs=rvmv[:],
                    start=True, stop=True)
                nc.scalar.copy(w0e_sb[:, jt, 87:88], w0c_ps[:, jt:jt + 1])

            nc.vector.memset(rv_ext[64:128, :], 1.0)
            nc.vector.tensor_copy(rv_ext[0:64, :], mrk[0:64, 3:4])
            # rows 64..85 must still be rV — rewrite them via a 32-aligned op
            nc.vector.tensor_copy(rv_ext[64:86, :], mrk[64:86, 3:4])

            # ------------- softmax -------------
            for it in range(2):
                log_ps = psB.tile([128, C], f32, tag="psB", name=f"lg{it}")
                nc.tensor.matmul(
                    log_ps[:], lhsT=eqtL_sb[:, it * 128:(it + 1) * 128],
                    rhs=u_sb[:], start=True, stop=True)
                rmax = small.tile([128, 1], f32, tag="rmax", name=f"rm{it}")
                nc.vector.reduce_max(rmax[:], log_ps[:], axis=AX.X)
                nbias = small.tile([128, 1], f32, tag="nbias", name=f"nb{it}")
                nc.vector.tensor_scalar_mul(nbias[:], rmax[:], -SCALE)
                a_bf = small.tile([128, C], bf16, tag=f"abf{it}",
                                  name=f"ab{it}")
                nc.scalar.activation(
                    out=a_bf[:], in_=log_ps[:], func=AF.Exp,
                    bias=nbias[:], scale=SCALE, accum_out=z2[:, it:it + 1])
                nc.vector.reciprocal(recip2[:, it:it + 1], z2[:, it:it + 1])
                an = small.tile([128, C], bf16, tag=f"attn{it}",
                                name=f"an{it}")
                nc.vector.tensor_scalar(
                    out=an[:], in0=a_bf[:], scalar1=recip2[:, it:it + 1],
                    scalar2=None, op0=OP.mult)
                att_n.append(an)

        # ------------- NT: ntc = w0e^T @ att_n^T -------------
        with tc.tile_pool(name="psC", bufs=3, space="PSUM") as psC:
            ntc_ps = psC.tile([89, C], f32, tag="psC", name="ntc")
            for jt in range(2):
                at_ps = psC.tile([128, C], bf16, tag="psC", name=f"atp{jt}")
                for it in range(2):
                    nc.tensor.transpose(
                        at_ps[:, it * 128:(it + 1) * 128],
                        att_n[it][:, jt * 128:(jt + 1) * 128],
                        identb[:])
                at_bf = small.tile([128, C], bf16, tag=f"atbf{jt}",
                                   name=f"atb{jt}")
                nc.scalar.copy(at_bf[:], at_ps[:])
                nc.tensor.matmul(
                    ntc_ps[:], lhsT=w0e_sb[:, jt, :], rhs=at_bf[:],
                    start=(jt == 0), stop=(jt == 1))

            nc.scalar.activation(out=lhs_m2[:], in_=ntc_ps[:], func=AF.Copy,
                                 scale=rv_ext[0:89, :])

        # ------------- m2 + epilogue -------------
        osml = ctx.enter_context(tc.tile_pool(name="osml", bufs=3))
        psD = ctx.enter_context(tc.tile_pool(name="psD", bufs=3, space="PSUM"))
        for it in range(2):
            for ch in range(16):
                ostg = osml.tile([128, 2, 512], bf16, tag="ostg", bufs=3,
                                 name=f"o{it}{ch}")
                o_ps = psD.tile([128, 2, 512], f32, tag="psD",
                                name=f"op{it}{ch}")
                for h in range(2):
                    nc.tensor.matmul(
                        o_ps[:, h, :],
                        lhsT=lhs_m2[0:89, it * 128:(it + 1) * 128],
                        rhs=rhs_m2[0:89,
                                   (2 * ch + h) * 512:(2 * ch + h + 1) * 512],
                        start=True, stop=True)
                xr2 = x_sb[:, it, ch // 4, (ch % 4) * 2:(ch % 4) * 2 + 2, :]
                nc.vector.tensor_tensor(
                    out=ostg[:], in0=o_ps[:], in1=xr2, op=OP.add)
                deng = nc.sync if (ch % 2 == 0) else nc.scalar
                deng.dma_start(
                    out=out_d[it * 128:(it + 1) * 128,
                              ch * 1024:(ch + 1) * 1024],
                    in_=ostg[:])

    nc.compile()
    return nc


def _host_prep(x, gamma, beta, w_qkv, b_qkv):
    xf = np.ascontiguousarray(np.asarray(x, np.float32).reshape(B * C, S))
    gam = np.asarray(gamma, np.float32).reshape(-1)
    bet = np.asarray(beta, np.float32).reshape(-1)
    w_qkv = np.asarray(w_qkv, np.float32)
    b_qkv = np.asarray(b_qkv, np.float32)
    w_q, w_k, w_v = w_qkv[:C], w_qkv[C:2 * C], w_qkv[2 * C:]
    b_q, b_k, b_v = b_qkv[:C], b_qkv[C:2 * C], b_qkv[2 * C:]

    ii = np.arange(C)
    ekt = np.zeros((86, C), np.float32)
    ekt[(C + ii) // 3 - 85, ii] = w_k
    eqtL = np.zeros((88, C), np.float32)
    eqtL[ii // 3, ii] = w_q
    eqtL[86] = b_q
    eqtL[87] = w_q
    w0 = np.zeros((C, 87), np.float32)
    w0[ii, (2 * C + ii) // 3 - 170] = w_v
    w0[:, 86] = b_v
    w0e = np.zeros((C, 89), np.float32)
    w0e[:, 0:87] = w0
    w0e[:, 88] = w_v              # c2 column = rowsum of w0[:, 0:86]
    w0e = w0e.astype(_BF)
    w0t = np.ascontiguousarray(w0[:, 0:86].T).astype(_BF)   # [86, C]

    G1, G2 = gam.sum(), (gam * gam).sum()
    Gb = (gam * bet).sum()
    B1, B2 = bet.sum(), (bet * bet).sum()
    sc = np.zeros((1, 8), np.float32)
    sc[0, :6] = [G1, G2, Gb, B1, -Gb, -G1]

    crows = np.stack([w_k, b_k, float(S) * b_k, B1 * b_k + B2 * w_k],
                     0).astype(np.float32)

    idb = np.eye(128, dtype=np.float32).astype(_BF)

    in_maps = []
    for r in range(NCORES):
        b = r // 2
        h = r % 2
        sl = slice(h * SHH, (h + 1) * SHH)
        gl = gam[sl]
        bl = bet[sl]

        xs_bf = xf[b * C:(b + 1) * C, sl].astype(_BF)
        gx = (xf[b * C:(b + 1) * C, sl] * gl[None, :]).astype(np.float32)
        # xst partition-major, gamma-prescaled: [g*x_A | g, b, 1 | g*x_K]
        xst = np.empty((128, NST, UTW), _BF)
        xst[:, :, 0:86] = gx[0:86, :].reshape(86, NST, 128).transpose(2, 1, 0)
        xst[:, :, 86] = gl.reshape(NST, 128).T
        xst[:, :, 87] = bl.reshape(NST, 128).T
        xst[:, :, 88] = 1.0
        xst[:, :, 89:175] = gx[85:171, :].reshape(86, NST, 128).transpose(2, 1, 0)

        rhsm = np.empty((89, SHH), np.float32)
        rhsm[0:86] = gx[170:256, :]
        rhsm[86] = 1.0
        rhsm[87] = -gl
        rhsm[88] = bl

        in_maps.append({
            "xs": xs_bf,
            "xst": xst.reshape(128, NST * UTW),
            "rhsm": rhsm.astype(_BF),
            "ekt": ekt,
            "eqtL": eqtL,
            "w0e": w0e,
            "w0t": w0t,
            "crows": crows,
            "idb": idb,
            "sc": sc,
        })
    return in_maps


def kernel(x, gamma, beta, w_qkv, b_qkv):
    from concourse.bass_utils import run_bass_kernel_spmd

    if "nc" not in _cache:
        _cache["nc"] = _build_program()
    nc = _cache["nc"]

    in_maps = _host_prep(x, gamma, beta, w_qkv, b_qkv)
    res = run_bass_kernel_spmd(nc, in_maps, core_ids=list(range(NCORES)))
    out = np.empty((B * C, S), np.float32)
    for r in range(NCORES):
        b = r // 2
        h = r % 2
        out[b * C:(b + 1) * C, h * SHH:(h + 1) * SHH] = res.results[r]["out"]
    return out.reshape(np.asarray(x).shape)


if __name__ == "__main__":
    rng = np.random.default_rng(0)
    inputs = {
        "x": rng.standard_normal((B, C, 32, 32, 32)).astype(np.float32),
        "gamma": (1 + 0.1 * rng.standard_normal((32, 32, 32))).astype(np.float32),
        "beta": (0.1 * rng.standard_normal((32, 32, 32))).astype(np.float32),
        "w_qkv": (0.5 * rng.standard_normal(3 * C)).astype(np.float32),
        "b_qkv": (0.05 * rng.standard_normal(3 * C)).astype(np.float32),
    }
    o = kernel(**inputs)
    print("out", o.shape, o.dtype, float(np.abs(o).mean()))
